# revision 42
# baseline (speedup 1.0000x reference)
"""CGMM layer-0 forward on 8 Trainium2 NeuronCores.

Math: per-node likelihood depends only on the node's discrete label
x[n] in [0, 32), so
    lik_node[n, :] = L[x[n], :]         with L a (32, 16) table
    lik_graph[s,:] = sum_m count[s, m] * L[m, :]
where count[s, m] = #{nodes of graph s with label m}.

Layout: TRANSPOSED node-slot layout — one SBUF column per graph, the
graph's node labels stacked along the 128 partitions.  Graphs are
pre-partitioned across the 8 cores (625 each, zero cross-core traffic)
and size-sorted per core; nodes 128.. of oversized graphs go to mirror
columns whose counts are psum-accumulated onto the parent group.

Radix-packed histogram: host splits each label x = 11*t + m (t in
0..2, m in 0..10) and ships xlo = m and s = 256^t.  One DVE
tensor_scalar_ptr per m computes [xlo == m] * s (4x bf16 perf mode),
and a PE matmul per 4-graph group colsums it into
    S[m] = c0 + 256*c1 + 65536*c2      (exact in fp32, < 2^24)
so 11 planes replace 32.  The decode (mod/scale chains) runs on tiny
[44, 157] tiles, with all scale factors folded into three L-table
variants contracted against the three count sections.

L table computed on device from B, Pi; the -ln spi[g] normalizer is
folded into every L row (sum_m count[s, m] = size[s]).
"""

import math

import numpy as np

N_NODES = 500_000
N_GRAPHS = 5_000
C = 16
M = 32
G = 16
N_CORES = 8
GPC = N_GRAPHS // N_CORES  # graphs per core = 625
SCOL = 628                 # main graph columns (625 + pad to 4*157)
OVW = 8                    # mirror columns for graphs > 128 nodes
W = SCOL + OVW             # total columns = 636
NGRP = SCOL // 4           # 4-graph count groups = 157
NGRP_ALL = W // 4          # incl. 2 mirror groups = 159
TCAP = 128                 # partitions per column
TUSE = 127                 # used slots (<=127 keeps digit fractions < 0.5)
PAD_LABEL = 64.0
R = 11                     # labels per radix digit; planes m in [0, R)
NSEC = 44                  # count-section rows = R * 4


def _build_nc():
    import concourse.bass as bass
    import concourse.bacc as bacc
    import concourse.tile as tile
    import concourse.mybir as mybir

    fp32 = mybir.dt.float32
    bf16 = mybir.dt.bfloat16
    i32 = mybir.dt.int32
    Alu = mybir.AluOpType
    Act = mybir.ActivationFunctionType

    nc = bacc.Bacc("TRN2", target_bir_lowering=False, debug=False)

    xl_d = nc.dram_tensor("xl", [TCAP, W], bf16, kind="ExternalInput").ap()
    xs_d = nc.dram_tensor("xs", [TCAP, W], bf16, kind="ExternalInput").ap()
    # par = B (C, G*M g-major) concat Pi (C, G) along the free dim, fp32
    par_d = nc.dram_tensor("par", [C, G * M + G], fp32, kind="ExternalInput").ap()
    # bf16 copy of B for the 4x-mode eb product
    bbf_d = nc.dram_tensor("bbf", [C, G * M], bf16, kind="ExternalInput").ap()
    # constant selection matrices building the three block-diagonal
    # L-section tables on the PE (radix scales folded in)
    ee_d = nc.dram_tensor("ee", [M, 12 * NSEC], bf16, kind="ExternalInput").ap()
    out_d = nc.dram_tensor("out", [SCOL, G], fp32, kind="ExternalOutput").ap()

    with tile.TileContext(nc) as tc:
        with (
            tc.tile_pool(name="main", bufs=1) as main,
            tc.tile_pool(name="psA", bufs=1, space="PSUM") as psA,
            tc.tile_pool(name="psB", bufs=1, space="PSUM") as psB,
        ):
            # ---- input DMAs (XL/XS first on their queues: planes need both) ----
            XL = main.tile([TCAP, W], bf16)
            nc.sync.dma_start(out=XL, in_=xl_d)
            XS = main.tile([TCAP, W], bf16)
            nc.gpsimd.dma_start(out=XS, in_=xs_d)
            Par = main.tile([C, G * M + G], fp32)
            nc.scalar.dma_start(out=Par, in_=par_d)
            Bbf = main.tile([C, G * M], bf16)
            nc.gpsimd.dma_start(out=Bbf, in_=bbf_d)
            EE = main.tile([M, 12 * NSEC], bf16)
            nc.sync.dma_start(out=EE, in_=ee_d)
            Bt = Par[:, 0 : G * M]
            Pit = Par[:, G * M : G * M + G]

            # ---- tiny constants (Pool, during DMA window) ----
            onesb = main.tile([TCAP, 1], bf16)
            nc.gpsimd.memset(onesb, 1.0)
            onesc = main.tile([C, 1], fp32)
            nc.gpsimd.memset(onesc, 1.0)
            onesm = main.tile([1, M], fp32)
            nc.gpsimd.memset(onesm, 1.0)
            zerocg = main.tile([C, G], fp32)
            nc.gpsimd.memset(zerocg, 0.0)

            # ---- ACT: exponentials ----
            expPi = main.tile([C, G], fp32)
            nc.scalar.activation(expPi, Pit, Act.Exp)
            expB = main.tile([C, G * M], bf16)
            nc.scalar.activation(expB, Bt, Act.Exp)

            # ---- radix bit-planes H[t, m, w] = [xl[t, w] == m] (4x DVE),
            # then P4[t, g, m, u] = H * s via two 2x tensor_tensor mults
            # (DVE + Pool slices in parallel; (m, u) interleave keeps each
            # count-group's stationary one contiguous free dim) ----
            H = main.tile([TCAP, R, W], bf16)
            P4 = main.tile([TCAP, NGRP_ALL, R, 4], bf16)
            Ho = H.rearrange("t m (g u) -> t g m u", u=4)
            XSb = (
                XS.rearrange("t (g u) -> t g u", u=4)
                .unsqueeze(2)
                .broadcast_to((TCAP, NGRP_ALL, R, 4))
            )

            def plane(m):
                nc.vector.tensor_scalar(
                    out=H[:, m, :], in0=XL, scalar1=float(m), scalar2=0.0,
                    op0=Alu.is_equal, op1=Alu.add,
                )

            DSPL = 59  # DVE multiplies groups [0, DSPL) in one batch;
            # Pool handles the rest with one TT per plane, overlapped with
            # the DVE planes still streaming

            def pool_mult(m):
                nc.gpsimd.tensor_tensor(
                    out=P4[:, DSPL:NGRP_ALL, m, :], in0=Ho[:, DSPL:NGRP_ALL, m, :],
                    in1=XSb[:, DSPL:NGRP_ALL, m, :], op=Alu.mult,
                )

            def mults():
                nc.vector.tensor_tensor(
                    out=P4[:, 0:DSPL], in0=Ho[:, 0:DSPL],
                    in1=XSb[:, 0:DSPL], op=Alu.mult,
                )

            plane(0)
            pool_mult(0)
            plane(1)
            pool_mult(1)

            # ---- L-table, slim form ----
            # w[c,g] = expPi / sumB = exp(Pi - ln sumB);  lnw = Pi - ln sumB
            #   L[m,g] = ZP/Z - ln spi[g]
            #   Z[m,g]  = sum_c expB * w
            #   ZP[m,g] = sum_c (B*expB) * w + sum_c expB * (w * ln w)
            # sumB via a Pool add-tree (keeps the whole chain off DVE and
            # lets it start ~3us earlier than the scheduler would allow)
            ebv = expB.rearrange("c (g m) -> c g m", m=M)
            t16 = main.tile([C, G, 16], fp32)
            nc.gpsimd.tensor_tensor(
                out=t16, in0=ebv[:, :, 0:M:2], in1=ebv[:, :, 1:M:2], op=Alu.add
            )
            t8 = main.tile([C, G, 8], fp32)
            nc.gpsimd.tensor_tensor(
                out=t8, in0=t16[:, :, 0:16:2], in1=t16[:, :, 1:16:2], op=Alu.add
            )
            t4 = main.tile([C, G, 4], fp32)
            nc.gpsimd.tensor_tensor(
                out=t4, in0=t8[:, :, 0:8:2], in1=t8[:, :, 1:8:2], op=Alu.add
            )
            t2 = main.tile([C, G, 2], fp32)
            nc.gpsimd.tensor_tensor(
                out=t2, in0=t4[:, :, 0:4:2], in1=t4[:, :, 1:4:2], op=Alu.add
            )
            sumB = main.tile([C, G, 1], fp32)
            nc.gpsimd.tensor_tensor(
                out=sumB, in0=t2[:, :, 0:1], in1=t2[:, :, 1:2], op=Alu.add,
            )
            # lnw = Pi - ln(sumB); w = exp(lnw)
            lnSumB = main.tile([C, G], fp32)
            nc.scalar.activation(lnSumB, sumB.rearrange("c g one -> c (g one)"), Act.Ln)
            lnw = main.tile([C, G], fp32)
            nc.gpsimd.tensor_tensor(out=lnw, in0=Pit, in1=lnSumB, op=Alu.subtract)
            w = main.tile([C, G], fp32)
            nc.scalar.activation(w, lnw, Act.Exp)
            wb = main.tile([C, G], bf16)
            with nc.allow_low_precision(reason="bf16 posterior weights"):
                nc.gpsimd.tensor_tensor(out=wb, in0=w, in1=zerocg, op=Alu.add)
            w2b = main.tile([C, G], bf16)
            with nc.allow_low_precision(reason="bf16 posterior weights"):
                nc.gpsimd.tensor_tensor(out=w2b, in0=wb, in1=lnw, op=Alu.mult)

            plane(2)
            pool_mult(2)

            # eb = B * expB on Pool (TensorTensor mult is Pool-legal)
            eb = main.tile([C, G * M], bf16)
            nc.gpsimd.tensor_tensor(out=eb, in0=Bbf, in1=expB, op=Alu.mult)

            plane(3)
            pool_mult(3)
            plane(4)
            pool_mult(4)

            # spi = sum_c expPi (PE), lnspi, broadcast to [M, G] (PE)
            ps_misc = psA.tile([M, 2, G], fp32)
            spi = ps_misc[0:1, 0, :]
            nc.tensor.matmul(spi, onesc[:, 0:1], expPi, start=True, stop=True)
            lnspi = main.tile([1, G], fp32)
            nc.scalar.activation(lnspi, spi, Act.Ln)
            lnspiM = ps_misc[:, 1, :]
            nc.tensor.matmul(lnspiM, onesm, lnspi, start=True, stop=True)

            # Z, ZP via per-g contractions over c (g-major layout)
            ps_zzp = psA.tile([M, 2, G], fp32)
            z32 = ps_zzp[:, 0, :]
            zp32 = ps_zzp[:, 1, :]
            for g in range(G):
                bg = expB[:, g * M : (g + 1) * M]
                nc.tensor.matmul(
                    z32[:, g : g + 1], bg, wb[:, g : g + 1], start=True, stop=True
                )
                nc.tensor.matmul(
                    zp32[:, g : g + 1], eb[:, g * M : (g + 1) * M],
                    wb[:, g : g + 1], start=True, stop=False,
                )
                nc.tensor.matmul(
                    zp32[:, g : g + 1], bg, w2b[:, g : g + 1],
                    start=False, stop=True,
                )

            plane(5)
            pool_mult(5)
            plane(6)
            pool_mult(6)

            # stage psum results to SBUF on ACT so Pool can finish the
            # L table without touching DVE
            zzps = main.tile([M, 2, G], fp32)
            nc.scalar.copy(zzps, ps_zzp)
            lnspiMs = main.tile([M, G], fp32)
            nc.scalar.copy(lnspiMs, lnspiM)
            rz32 = main.tile([M, G], fp32)
            L0 = main.tile([M, G], fp32)
            Ltn = main.tile([M, G], bf16)
            with tc.high_priority():
                nc.vector.reciprocal(rz32, zzps[:, 0, :])
            nc.gpsimd.tensor_tensor(out=L0, in0=zzps[:, 1, :], in1=rz32, op=Alu.mult)
            # Ltn = -L = lnspi - ZP/Z  (negation folded)
            with nc.allow_low_precision(reason="bf16 L table"):
                nc.gpsimd.tensor_tensor(
                    out=Ltn, in0=lnspiMs, in1=L0, op=Alu.subtract
                )

            for m in range(7, R):
                plane(m)
                pool_mult(m)
            mults()

            # block-diag L sections on PE while planes stream:
            # Lsec[sec][p=4m+u, (u', gen)] = scale_sec * Ltn[sel(m), gen]
            # for u'==u, via constant selection matmuls
            LsecP = psA.tile([NSEC, 3, 4, G], fp32)
            for sec in range(3):
                for u in range(4):
                    col = (sec * 4 + u) * NSEC
                    nc.tensor.matmul(
                        LsecP[:, sec, u, :], EE[:, col : col + NSEC],
                        Ltn, start=True, stop=True,
                    )
            LsecS = main.tile([NSEC, 3, 4, G], bf16)
            nc.scalar.copy(LsecS, LsecP)
            La = LsecS[:, 0, :, :]
            Lb = LsecS[:, 1, :, :]
            Lc = LsecS[:, 2, :, :]

            # ---- counts: one psum column per 4-graph group; mirror
            # groups accumulate onto their parent group's column ----
            CNTP = psB.tile([NSEC, NGRP], fp32)
            for g in range(NGRP):
                lhsT = P4[:, g, :, :].rearrange("t m u -> t (m u)")
                mirror = NGRP + g
                if mirror < NGRP_ALL:
                    nc.tensor.matmul(
                        CNTP[:, g : g + 1], lhsT, onesb, start=True, stop=False
                    )
                    lhsT2 = P4[:, mirror, :, :].rearrange("t m u -> t (m u)")
                    nc.tensor.matmul(
                        CNTP[:, g : g + 1], lhsT2, onesb, start=False, stop=True
                    )
                else:
                    nc.tensor.matmul(
                        CNTP[:, g : g + 1], lhsT, onesb, start=True, stop=True
                    )

            # ---- radix decode into three count sections (all exact) ----
            # S = c0 + 256 c1 + 65536 c2 with all c_i <= 127 (host caps
            # columns at 127 slots) so every S*2^-k has fraction < 0.5 and
            # the fp->int cast is exact under any rounding mode:
            #   c2 = int(S/2^16); q = int(S/2^8) = c1 + 256 c2
            #   c1 = q - 256 c2;  c0 = S - 256 q
            qi = main.tile([NSEC, NGRP], i32)
            nc.vector.tensor_scalar(
                out=qi, in0=CNTP, scalar1=2.0**-8, scalar2=0.0,
                op0=Alu.mult, op1=Alu.add,
            )
            c2i = main.tile([NSEC, NGRP], i32)
            nc.vector.tensor_scalar(
                out=c2i, in0=qi, scalar1=2.0**-8, scalar2=0.0,
                op0=Alu.mult, op1=Alu.add,
            )
            CNT_c = main.tile([NSEC, NGRP], bf16)
            nc.vector.tensor_scalar(
                out=CNT_c, in0=c2i, scalar1=0.0, scalar2=0.0,
                op0=Alu.add, op1=Alu.add,
            )
            CNT_b = main.tile([NSEC, NGRP], bf16)
            nc.vector.scalar_tensor_tensor(
                out=CNT_b, in0=CNT_c, scalar=-256.0, in1=qi,
                op0=Alu.mult, op1=Alu.add,
            )
            CNT_a = main.tile([NSEC, NGRP], bf16)
            nc.vector.scalar_tensor_tensor(
                out=CNT_a, in0=qi, scalar=-256.0, in1=CNTP,
                op0=Alu.mult, op1=Alu.add,
            )

            # ---- final: out[c, (u,gen)] = sum over the three sections ----
            ps_out = psA.tile([TCAP, 2, 4, G], fp32)
            OUT1 = ps_out[:, 0, :, :]
            OUT2 = ps_out[0 : NGRP - TCAP, 1, :, :]
            for ci, (lo, hi, OUT) in enumerate(
                [(0, TCAP, OUT1), (TCAP, NGRP, OUT2)]
            ):
                of = OUT.rearrange("c u g -> c (u g)")
                nc.tensor.matmul(
                    of, CNT_a[:, lo:hi], La.rearrange("p u g -> p (u g)"),
                    start=True, stop=False,
                )
                nc.tensor.matmul(
                    of, CNT_b[:, lo:hi], Lb.rearrange("p u g -> p (u g)"),
                    start=False, stop=False,
                )
                nc.tensor.matmul(
                    of, CNT_c[:, lo:hi], Lc.rearrange("p u g -> p (u g)"),
                    start=False, stop=True,
                )
            OUTS1 = main.tile([TCAP, 4, G], fp32)
            nc.vector.tensor_scalar(
                out=OUTS1, in0=OUT1, scalar1=0.0, scalar2=0.0,
                op0=Alu.add, op1=Alu.add,
            )
            OUTS2 = main.tile([NGRP - TCAP, 4, G], fp32)
            nc.scalar.copy(OUTS2, OUT2)
            ov = out_d.rearrange("(c u) g -> c u g", u=4)
            nc.sync.dma_start(out=ov[0:TCAP], in_=OUTS1)
            nc.scalar.dma_start(out=ov[TCAP:NGRP], in_=OUTS2)

    nc.compile()
    return nc


def _host_pack(x, batch):
    """Pack node labels into the transposed radix layout.

    Returns (XLs, XSs [N_CORES][TCAP, W] bf16, orders)."""
    import ml_dtypes

    sizes = np.bincount(batch, minlength=N_GRAPHS)
    T = max(32, int(math.ceil(sizes.max() / 16.0)) * 16)
    assert T - TUSE <= TUSE, "graph overflow exceeds one mirror column"
    xv = x.astype(np.int64)
    xlo_all = (xv % R).astype(np.float32)
    s_all = (256.0 ** (xv // R)).astype(np.float32)

    xp = np.full((N_GRAPHS, T), PAD_LABEL, dtype=np.float32)
    sp = np.zeros((N_GRAPHS, T), dtype=np.float32)
    mask = np.arange(T)[None, :] < sizes[:, None]
    # batch is sorted, so row-major True positions match x's node order
    xp[mask] = xlo_all
    sp[mask] = s_all

    XLs, XSs, orders = [], [], []
    for i in range(N_CORES):
        s = sizes[i * GPC : (i + 1) * GPC]
        order = np.argsort(-s, kind="stable")
        orders.append(order)
        xs_ = xp[i * GPC : (i + 1) * GPC][order]  # [GPC, T] size desc
        ss_ = sp[i * GPC : (i + 1) * GPC][order]
        n_ov = int((s > TUSE).sum())
        assert n_ov <= OVW, f"core {i}: {n_ov} oversized graphs > {OVW}"
        XL = np.full((TCAP, W), PAD_LABEL, dtype=np.float32)
        XS = np.zeros((TCAP, W), dtype=np.float32)
        XL[:TUSE, :GPC] = xs_[:, :TUSE].T
        XS[:TUSE, :GPC] = ss_[:, :TUSE].T
        if n_ov and T > TUSE:
            ovt = T - TUSE
            XL[:ovt, SCOL : SCOL + n_ov] = xs_[:n_ov, TUSE:T].T
            XS[:ovt, SCOL : SCOL + n_ov] = ss_[:n_ov, TUSE:T].T
        XLs.append(np.ascontiguousarray(XL.astype(ml_dtypes.bfloat16)))
        XSs.append(np.ascontiguousarray(XS.astype(ml_dtypes.bfloat16)))
    return XLs, XSs, orders


def _host_par(B, Pi):
    import ml_dtypes

    # B (C, M, G) -> g-major flat (C, G*M)
    Bgm = np.ascontiguousarray(np.transpose(B, (0, 2, 1)).reshape(C, G * M))
    par = np.ascontiguousarray(
        np.concatenate([Bgm, Pi], axis=1).astype(np.float32)
    )
    bbf = np.ascontiguousarray(Bgm.astype(ml_dtypes.bfloat16))
    # selection matrices: Lsec[sec][4m+u, u] = scale*Ltn[sel] needs
    # ee[sel(m), (sec,u)-block col 4m+u] = scale_sec
    ee = np.zeros((M, 12 * NSEC), dtype=np.float32)
    for sec, (off, scale) in enumerate([(0, 1.0), (R, 1.0), (2 * R, 1.0)]):
        for u in range(4):
            base = (sec * 4 + u) * NSEC
            for m in range(R):
                if off + m < M:
                    ee[off + m, base + 4 * m + u] = scale
    ee = np.ascontiguousarray(ee.astype(ml_dtypes.bfloat16))
    return par, bbf, ee


def kernel(x, edge_index, batch, B, Pi):
    from concourse.bass_utils import run_bass_kernel_spmd

    x = np.asarray(x).astype(np.int64)
    batch = np.asarray(batch).astype(np.int64)
    B = np.asarray(B, dtype=np.float32)
    Pi = np.asarray(Pi, dtype=np.float32)

    XLs, XSs, orders = _host_pack(x, batch)
    par, bbf, ee = _host_par(B, Pi)

    nc = _build_nc()

    in_maps = [
        {"xl": XLs[i], "xs": XSs[i], "par": par, "bbf": bbf, "ee": ee}
        for i in range(N_CORES)
    ]

    res = run_bass_kernel_spmd(
        nc, in_maps, core_ids=list(range(N_CORES)), **_RUN_KWARGS
    )
    kernel.last_results = res
    parts = []
    for i in range(N_CORES):
        o_sorted = res.results[i]["out"][:GPC]
        o = np.empty_like(o_sorted)
        o[orders[i]] = o_sorted
        parts.append(o)
    out = np.concatenate(parts)
    return out[:, None, :].astype(np.float32)


# test harnesses may set extra run kwargs (e.g. trace) here
_RUN_KWARGS = {}
